# revision 29
# baseline (speedup 1.0000x reference)
"""Multi-head causal attention (B=4, T=2048, C=1024, H=16) on 8 TRN2 cores.

Sharding: head-parallel. Each core owns 2 heads for all 4 batches:
  - QKV projection: x (replicated, pre-transposed + bf16-cast on host to
    [C, B*T]) @ per-core bf16 weight column slices -> q/k/v in
    [head_dim, tokens] layout (f32 PSUM accumulation).
  - k and v are ALSO evicted to f32 staging tiles and DMA'd out directly
    (they are reference outputs; f32 eviction keeps them at full PSUM
    accuracy instead of bf16).
  - V is PE-transposed to [tokens, head_dim] bf16 with a fused ones
    column, so each PV matmul also produces the softmax denominators.
  - Scores are computed transposed (k on partitions, q on free dim);
    softmax uses exp without max-subtraction (scores ~N(0,1) after the
    1/sqrt(D) scale; no overflow risk). Causal masking: skip invalid
    k-tiles, narrow the q range on diagonal tiles, and multiply the
    [128,128] diagonal block by a triangle mask.
  - Normalization: reciprocal of the fused denominators in f32, DMA
    partition-broadcast, multiply into the context (f32) -> bf16 c2.
  - Output projection is row-parallel; the 8 partial [C, B*T] f32
    outputs are summed on the host.
"""

import ml_dtypes
import numpy as np

import concourse.bass as bass
import concourse.mybir as mybir
import concourse.tile as tile
from concourse import bacc
from concourse.bass_utils import run_bass_kernel_spmd
from concourse.masks import make_identity

B, T, C = 4, 2048, 1024
H, D = 16, 64
NCORE = 8
HPC = H // NCORE          # heads per core = 2
HD = HPC * D              # 128 head-dim columns per core
TOK = B * T               # 8192
QT = 512                  # query tile (free dim)
KT = 128                  # key tile (partitions)
NSL = TOK // KT           # 64 key slices (global, across batches)
P = 128
CS = C // P               # 8 contraction slices for the projections
F32 = mybir.dt.float32
BF16 = mybir.dt.bfloat16
EXP = mybir.ActivationFunctionType.Exp
SCALE = 1.0 / 8.0         # 1/sqrt(D)
VW = 66                   # per-head stride in vsb: 64 v cols + ones + pad

CFG = {
    "xt_split": 1,
    "sp_bufs": 2,
    "cx_bufs": 1,
    "ob_bufs": 2,
    "pt_bufs": 3,
    "ot_bufs": 2,
    "xp_bufs": 3,
}


def _plan(mask2d):
    """Per q-tile schedule of k-tiles from the [T, T] bool mask.

    Entry: (ki, qlo, tri_j, gen_idx)
      ki      key-tile index within the batch
      qlo     first valid q column within the 512-wide q tile (causal narrowing)
      tri_j   not None -> multiply cols [qlo, qlo+128) by the shared triangle
      gen_idx not None -> multiply full tile by mask_tiles[gen_idx]
    """
    causal = bool(mask2d[np.triu_indices(T, 1)].sum() == 0) and bool(
        np.tril(np.ones((T, T), bool))[mask2d ^ True].sum() == 0
    )
    plans = []
    gen_tiles = []
    cache = {}
    for qi in range(T // QT):
        qs = qi * QT
        ents = []
        for ki in range(T // KT):
            blk = mask2d[qs : qs + QT, ki * KT : (ki + 1) * KT]
            if not blk.any():
                continue
            if blk.all():
                ents.append((ki, 0, None, None))
            elif causal:
                j = ki - qs // KT
                assert 0 <= j < QT // KT
                ents.append((ki, KT * j, j, None))
            else:
                key = blk.tobytes()
                if key not in cache:
                    cache[key] = len(gen_tiles)
                    gen_tiles.append(np.ascontiguousarray(blk.T).astype(np.float32))
                ents.append((ki, 0, None, cache[key]))
        plans.append(ents)
    mt = np.stack(gen_tiles) if gen_tiles else np.zeros((1, KT, QT), np.float32)
    return plans, mt


def _body(tc, nc, plans, n_mt, xT, wq, wk, wv, wo, mt, tri_in, outT, kTo, vTo):
    from contextlib import ExitStack

    with ExitStack() as ctx:
        const = ctx.enter_context(tc.tile_pool(name="const", bufs=1))
        qT = const.tile([P, TOK], BF16)        # rows 0:64 head0, 64:128 head1
        kTt = const.tile([P, TOK], BF16)
        vsb = const.tile([P, NSL, 2 * VW], BF16)  # per slice: v0|1|0|v1|1|0
        wq_sb = const.tile([P, CS, HD], BF16)
        wk_sb = const.tile([P, CS, HD], BF16)
        wv_sb = const.tile([P, CS, HD], BF16)
        wo_sb = const.tile([P, C], BF16)
        tri = const.tile([P, P], BF16)
        mts = const.tile([P, n_mt, QT], BF16)
        ident = const.tile([P, P], BF16)

        nc.sync.dma_start(wq_sb, wq.rearrange("(n p) m -> p n m", p=P))
        nc.sync.dma_start(wk_sb, wk.rearrange("(n p) m -> p n m", p=P))
        nc.sync.dma_start(wv_sb, wv.rearrange("(n p) m -> p n m", p=P))
        nc.scalar.dma_start(wo_sb, wo)
        nc.scalar.dma_start(mts, mt.rearrange("n p m -> p n m"))
        nc.scalar.dma_start(tri, tri_in)
        make_identity(nc, ident)
        nc.vector.memset(vsb[:, :, D : D + 1], 1.0)
        nc.vector.memset(vsb[:, :, D + 1 : D + 2], 0.0)
        nc.vector.memset(vsb[:, :, VW + D : VW + D + 1], 1.0)
        nc.vector.memset(vsb[:, :, VW + D + 1 : VW + D + 2], 0.0)

        # ---- QKV projection (all outputs transposed: [dims, tokens]) ----
        xTr = xT.rearrange("(n p) m -> p n m", p=P)
        with (
            tc.tile_pool(name="xp", bufs=CFG["xp_bufs"]) as xpool,
            tc.tile_pool(name="pps", bufs=2, space="PSUM") as pps,
            tc.tile_pool(name="tps", bufs=2, space="PSUM") as tps,
        ):
            _qkv_phase(
                tc, nc, xTr, xpool, pps, tps, wq_sb, wk_sb, wv_sb,
                qT, kTt, vsb, ident, kTo, vTo,
            )

        # ---- attention + output projection ----
        sp_pool = ctx.enter_context(tc.tile_pool(name="sp", bufs=CFG["sp_bufs"], space="PSUM"))
        cx_pool = ctx.enter_context(tc.tile_pool(name="cxp", bufs=CFG["cx_bufs"], space="PSUM"))
        ob_pool = ctx.enter_context(tc.tile_pool(name="obp", bufs=CFG["ob_bufs"], space="PSUM"))
        ppool = ctx.enter_context(tc.tile_pool(name="pp", bufs=CFG["pt_bufs"]))
        c2pool = ctx.enter_context(tc.tile_pool(name="c2p", bufs=2))
        otpool = ctx.enter_context(tc.tile_pool(name="otp", bufs=CFG["ot_bufs"]))
        smpool = ctx.enter_context(tc.tile_pool(name="smp", bufs=2))
        _attn_phase(
            tc, nc, plans, qT, kTt, vsb, wo_sb, tri, mts,
            sp_pool, cx_pool, ob_pool, ppool, c2pool, otpool, smpool, outT,
        )


def _qkv_phase(
    tc, nc, xTr, xpool, pps, tps, wq_sb, wk_sb, wv_sb, qT, kTt, vsb, ident, kTo, vTo
):
    for tt in range(TOK // QT):
        xt = xpool.tile([P, CS, QT], BF16, tag="xt")
        nsplit = 8 if tt == 0 else CFG["xt_split"]
        step = CS // nsplit
        for s in range(nsplit):
            nc.sync.dma_start(
                xt[:, s * step : (s + 1) * step, :],
                xTr[:, s * step : (s + 1) * step, tt * QT : (tt + 1) * QT],
            )
        tsl = slice(tt * QT, (tt + 1) * QT)
        for wsb, dst, out32 in (
            (wq_sb, qT, None),
            (wk_sb, kTt, kTo),
            (wv_sb, None, vTo),
        ):
            ps = pps.tile([P, QT], F32, tag="proj")
            for cs in range(CS):
                nc.tensor.matmul(
                    ps,
                    lhsT=wsb[:, cs, :],
                    rhs=xt[:, cs, :],
                    start=(cs == 0),
                    stop=(cs == CS - 1),
                )
            if dst is not None:
                nc.vector.tensor_copy(dst[:, tsl], ps)
                if out32 is not None:
                    nc.sync.dma_start(out32[:, tsl], dst[:, tsl])
            else:
                vt = xpool.tile([P, QT], BF16, tag="vt")
                nc.vector.tensor_copy(vt, ps)
                if out32 is not None:
                    nc.sync.dma_start(out32[:, tsl], vt)
                for sub in range(QT // P):
                    g = tt * (QT // P) + sub
                    tp = tps.tile([P, P], BF16, tag="tp")
                    nc.tensor.transpose(tp, vt[:, sub * P : (sub + 1) * P], ident)
                    nc.vector.tensor_copy(vsb[:, g, 0:D], tp[:, 0:D])
                    nc.vector.tensor_copy(vsb[:, g, VW : VW + D], tp[:, D:HD])


def _attn_phase(
    tc, nc, plans, qT, kTt, vsb, wo_sb, tri, mts,
    sp_pool, cx_pool, ob_pool, ppool, c2pool, otpool, smpool, outT,
):
    order = [(b, qi) for b in range(B) for qi in range(len(plans))]
    if CFG.get("qt_order") == "desc":
        order = [(b, qi) for b in range(B) for qi in reversed(range(len(plans)))]
    elif CFG.get("qt_order") == "mix":
        order = []
        for b in range(B):
            idx = list(range(len(plans)))
            hi, lo = idx[len(idx) // 2 :][::-1], idx[: len(idx) // 2]
            for a_, b_ in zip(hi, lo):
                order += [(b, a_), (b, b_)]
    for b, qi in order:
            ents = plans[qi]
            qoff = b * T + qi * QT
            cx = [
                cx_pool.tile([D + 2, QT], F32, tag=f"cx{h}", name=f"cx{h}")
                for h in range(HPC)
            ]
            n_e = len(ents)
            if n_e == 0:
                for h in range(HPC):
                    nc.vector.memset(cx[h], 0.0)
            for ei, (ki, qlo, j, mi) in enumerate(ents):
                g = b * (T // KT) + ki
                koff = g * KT
                sp = sp_pool.tile([P, HPC, QT], F32, tag="s", name="s")
                for h in range(HPC):
                    hs = h * D
                    nc.tensor.matmul(
                        sp[:, h, qlo:QT],
                        lhsT=kTt[hs : hs + D, koff : koff + KT],
                        rhs=qT[hs : hs + D, qoff + qlo : qoff + QT],
                        start=True,
                        stop=True,
                    )
                pt = ppool.tile([P, HPC, QT], BF16, tag="p", name="p")
                nc.scalar.activation(
                    pt[:, :, qlo:QT], sp[:, :, qlo:QT], EXP, scale=SCALE
                )
                for h in range(HPC):
                    if j is not None:
                        nc.gpsimd.tensor_mul(
                            pt[:, h, qlo : qlo + P], pt[:, h, qlo : qlo + P], tri
                        )
                    elif mi is not None:
                        nc.gpsimd.tensor_mul(pt[:, h, :], pt[:, h, :], mts[:, mi, :])
                    nc.tensor.matmul(
                        cx[h][:, qlo:QT],
                        lhsT=vsb[:, g, h * VW : (h + 1) * VW],
                        rhs=pt[:, h, qlo:QT],
                        start=(ei == 0),
                        stop=(ei == n_e - 1),
                        skip_group_check=True,
                    )

            c2 = c2pool.tile([P, QT], BF16, tag="c2")
            cxs = smpool.tile([D + 1, HPC, QT], F32, tag="cxs", name="cxs")
            for h in range(HPC):
                nc.vector.tensor_copy(cxs[:, h, :], cx[h][0 : D + 1, :])
            for h in range(HPC):
                rcp = smpool.tile([1, QT], F32, tag=f"r{h}", name=f"r{h}")
                nc.vector.reciprocal(rcp, cxs[D : D + 1, h, :])
                rb = smpool.tile([D, QT], F32, tag=f"rb{h}", name=f"rb{h}")
                nc.gpsimd.partition_broadcast(rb, rcp)
                nc.vector.tensor_mul(c2[h * D : (h + 1) * D, :], cxs[0:D, h, :], rb)
            ot = otpool.tile([P, CS, QT], F32, tag="ot")
            for os_ in range(C // P):
                op = ob_pool.tile([P, QT], F32, tag="ob", name="ob")
                nc.tensor.matmul(
                    op,
                    lhsT=wo_sb[:, os_ * P : (os_ + 1) * P],
                    rhs=c2,
                    start=True,
                    stop=True,
                )
                nc.vector.tensor_copy(ot[:, os_, :], op)
            nc.sync.dma_start(
                outT.rearrange("(n p) m -> p n m", p=P)[:, :, qoff : qoff + QT], ot
            )


def _build(plans, n_mt):
    nc = bacc.Bacc("TRN2", target_bir_lowering=False, debug=False)
    xT = nc.dram_tensor("xT", [C, TOK], BF16, kind="ExternalInput").ap()
    wq = nc.dram_tensor("wq", [C, HD], BF16, kind="ExternalInput").ap()
    wk = nc.dram_tensor("wk", [C, HD], BF16, kind="ExternalInput").ap()
    wv = nc.dram_tensor("wv", [C, HD], BF16, kind="ExternalInput").ap()
    wo = nc.dram_tensor("wo", [HD, C], BF16, kind="ExternalInput").ap()
    mt = nc.dram_tensor("mt", [n_mt, KT, QT], BF16, kind="ExternalInput").ap()
    tri_in = nc.dram_tensor("tri", [P, P], BF16, kind="ExternalInput").ap()
    outT = nc.dram_tensor("outT", [C, TOK], F32, kind="ExternalOutput").ap()
    kTo = nc.dram_tensor("kT", [HD, TOK], BF16, kind="ExternalOutput").ap()
    vTo = nc.dram_tensor("vT", [HD, TOK], BF16, kind="ExternalOutput").ap()
    with tile.TileContext(nc) as tc:
        _body(tc, nc, plans, n_mt, xT, wq, wk, wv, wo, mt, tri_in, outT, kTo, vTo)
    nc.compile()
    return nc


_CACHE = {}


def _get_nc(plans_key, plans, n_mt):
    if plans_key not in _CACHE:
        _CACHE[plans_key] = _build(plans, n_mt)
    return _CACHE[plans_key]


def _bf16(a):
    return np.asarray(a, np.float32).astype(ml_dtypes.bfloat16)


def _make_in_maps(x, w_qkv, w_o, mt):
    xT = _bf16(np.ascontiguousarray(x.reshape(TOK, C).T))
    tri = _bf16(np.triu(np.ones((P, P), np.float32)))  # tri[k, q] = 1 iff k <= q
    mtb = _bf16(mt)
    in_maps = []
    for c in range(NCORE):
        lo, hi = c * HD, (c + 1) * HD
        in_maps.append(
            {
                "xT": xT,
                "wq": _bf16(w_qkv[:, lo:hi]),
                "wk": _bf16(w_qkv[:, C + lo : C + hi]),
                "wv": _bf16(w_qkv[:, 2 * C + lo : 2 * C + hi]),
                "wo": _bf16(w_o[lo:hi, :]),
                "mt": mtb,
                "tri": tri,
            }
        )
    return in_maps


def _gather(results):
    out = np.zeros((C, TOK), np.float32)
    for r in results:
        out += r["outT"]
    out = np.ascontiguousarray(out.T).reshape(B, T, C)

    k = np.empty((B, H, T, D), np.float32)
    v = np.empty((B, H, T, D), np.float32)
    for c, r in enumerate(results):
        k[:, c * HPC : (c + 1) * HPC] = (
            r["kT"].astype(np.float32).reshape(HPC, D, B, T).transpose(2, 0, 3, 1)
        )
        v[:, c * HPC : (c + 1) * HPC] = (
            r["vT"].astype(np.float32).reshape(HPC, D, B, T).transpose(2, 0, 3, 1)
        )
    return out, k, v


def run(x, w_qkv, w_o, mask, trace=False):
    mask2d = np.asarray(np.broadcast_to(mask, (1, 1, T, T))).reshape(T, T).astype(bool)
    plans, mt = _plan(mask2d)
    plans_key = mask2d.tobytes()
    nc = _get_nc(plans_key, plans, mt.shape[0])
    in_maps = _make_in_maps(
        np.asarray(x, np.float32), np.asarray(w_qkv, np.float32),
        np.asarray(w_o, np.float32), mt,
    )
    res = run_bass_kernel_spmd(nc, in_maps, list(range(NCORE)), trace=trace)
    return _gather(res.results), res


def kernel(x, w_qkv, w_o, mask):
    (out, k, v), _ = run(x, w_qkv, w_o, mask)
    return out, k, v


# revision 33
# speedup vs baseline: 1.0405x; 1.0405x over previous
"""Multi-head causal attention (B=4, T=2048, C=1024, H=16) on 8 TRN2 cores.

Sharding: head-parallel. Each core owns 2 heads for all 4 batches:
  - QKV projection: x (replicated, pre-transposed + bf16-cast on host to
    [C, B*T]) @ per-core bf16 weight column slices -> q/k/v in
    [head_dim, tokens] layout (f32 PSUM accumulation).
  - k and v are ALSO evicted to f32 staging tiles and DMA'd out directly
    (they are reference outputs; f32 eviction keeps them at full PSUM
    accuracy instead of bf16).
  - V is PE-transposed to [tokens, head_dim] bf16 with a fused ones
    column, so each PV matmul also produces the softmax denominators.
  - Scores are computed transposed (k on partitions, q on free dim);
    softmax uses exp without max-subtraction (scores ~N(0,1) after the
    1/sqrt(D) scale; no overflow risk). Causal masking: skip invalid
    k-tiles, narrow the q range on diagonal tiles, and multiply the
    [128,128] diagonal block by a triangle mask.
  - Normalization: reciprocal of the fused denominators in f32, DMA
    partition-broadcast, multiply into the context (f32) -> bf16 c2.
  - Output projection is row-parallel; the 8 partial [C, B*T] f32
    outputs are summed on the host.
"""

import ml_dtypes
import numpy as np

import concourse.bass as bass
import concourse.mybir as mybir
import concourse.tile as tile
from concourse import bacc
from concourse.bass_utils import run_bass_kernel_spmd
from concourse.masks import make_identity

B, T, C = 4, 2048, 1024
H, D = 16, 64
NCORE = 8
HPC = H // NCORE          # heads per core = 2
HD = HPC * D              # 128 head-dim columns per core
TOK = B * T               # 8192
QT = 512                  # query tile (free dim)
KT = 128                  # key tile (partitions)
NSL = TOK // KT           # 64 key slices (global, across batches)
P = 128
CS = C // P               # 8 contraction slices for the projections
F32 = mybir.dt.float32
BF16 = mybir.dt.bfloat16
EXP = mybir.ActivationFunctionType.Exp
SCALE = 1.0 / 8.0         # 1/sqrt(D)
VW = 66                   # per-head stride in vsb: 64 v cols + ones + pad

CFG = {
    "xt_split": 1,
    "sp_bufs": 2,
    "cx_bufs": 1,
    "ob_bufs": 2,
    "pt_bufs": 8,
    "ot_bufs": 2,
    "xp_bufs": 3,
    "c2_bufs": 3,
    "sm_bufs": 4,
    "ot_split2": 1,
}


def _plan(mask2d):
    """Per q-tile schedule of k-tiles from the [T, T] bool mask.

    Entry: (ki, qlo, tri_j, gen_idx)
      ki      key-tile index within the batch
      qlo     first valid q column within the 512-wide q tile (causal narrowing)
      tri_j   not None -> multiply cols [qlo, qlo+128) by the shared triangle
      gen_idx not None -> multiply full tile by mask_tiles[gen_idx]
    """
    causal = bool(mask2d[np.triu_indices(T, 1)].sum() == 0) and bool(
        np.tril(np.ones((T, T), bool))[mask2d ^ True].sum() == 0
    )
    plans = []
    gen_tiles = []
    cache = {}
    for qi in range(T // QT):
        qs = qi * QT
        ents = []
        for ki in range(T // KT):
            blk = mask2d[qs : qs + QT, ki * KT : (ki + 1) * KT]
            if not blk.any():
                continue
            if blk.all():
                ents.append((ki, 0, None, None))
            elif causal:
                j = ki - qs // KT
                assert 0 <= j < QT // KT
                ents.append((ki, KT * j, j, None))
            else:
                key = blk.tobytes()
                if key not in cache:
                    cache[key] = len(gen_tiles)
                    gen_tiles.append(np.ascontiguousarray(blk.T).astype(np.float32))
                ents.append((ki, 0, None, cache[key]))
        plans.append(ents)
    mt = np.stack(gen_tiles) if gen_tiles else np.zeros((1, KT, QT), np.float32)
    return plans, mt


def _body(tc, nc, plans, n_mt, xT, wq, wk, wv, wo, mt, tri_in, outT, kTo, vTo):
    from contextlib import ExitStack

    with ExitStack() as ctx:
        const = ctx.enter_context(tc.tile_pool(name="const", bufs=1))
        qT = const.tile([P, TOK], BF16)        # rows 0:64 head0, 64:128 head1
        kTt = const.tile([P, TOK], BF16)
        vsb = const.tile([P, NSL, 2 * VW], BF16)  # per slice: v0|1|0|v1|1|0
        wq_sb = const.tile([P, CS, HD], BF16)
        wk_sb = const.tile([P, CS, HD], BF16)
        wv_sb = const.tile([P, CS, HD], BF16)
        wo_sb = const.tile([P, C], BF16)
        tri = const.tile([P, P], BF16)
        mts = const.tile([P, n_mt, QT], BF16)
        ident = const.tile([P, P], BF16)

        nc.sync.dma_start(wq_sb, wq.rearrange("(n p) m -> p n m", p=P))
        nc.sync.dma_start(wk_sb, wk.rearrange("(n p) m -> p n m", p=P))
        nc.sync.dma_start(wv_sb, wv.rearrange("(n p) m -> p n m", p=P))
        nc.scalar.dma_start(wo_sb, wo)
        nc.scalar.dma_start(mts, mt.rearrange("n p m -> p n m"))
        nc.scalar.dma_start(tri, tri_in)
        make_identity(nc, ident)
        nc.vector.memset(vsb[:, :, D : D + 1], 1.0)
        nc.vector.memset(vsb[:, :, D + 1 : D + 2], 0.0)
        nc.vector.memset(vsb[:, :, VW + D : VW + D + 1], 1.0)
        nc.vector.memset(vsb[:, :, VW + D + 1 : VW + D + 2], 0.0)

        # ---- QKV projection (all outputs transposed: [dims, tokens]) ----
        xTr = xT.rearrange("(n p) m -> p n m", p=P)
        with (
            tc.tile_pool(name="xp", bufs=CFG["xp_bufs"]) as xpool,
            tc.tile_pool(name="pps", bufs=2, space="PSUM") as pps,
            tc.tile_pool(name="tps", bufs=2, space="PSUM") as tps,
        ):
            _qkv_phase(
                tc, nc, xTr, xpool, pps, tps, wq_sb, wk_sb, wv_sb,
                qT, kTt, vsb, ident, kTo, vTo,
            )

        # ---- attention + output projection ----
        sp_pool = ctx.enter_context(tc.tile_pool(name="sp", bufs=CFG["sp_bufs"], space="PSUM"))
        cx_pool = ctx.enter_context(tc.tile_pool(name="cxp", bufs=CFG["cx_bufs"], space="PSUM"))
        ob_pool = ctx.enter_context(tc.tile_pool(name="obp", bufs=CFG["ob_bufs"], space="PSUM"))
        ppool = ctx.enter_context(tc.tile_pool(name="pp", bufs=CFG["pt_bufs"]))
        c2pool = ctx.enter_context(tc.tile_pool(name="c2p", bufs=CFG.get("c2_bufs", 2)))
        otpool = ctx.enter_context(tc.tile_pool(name="otp", bufs=CFG["ot_bufs"]))
        smpool = ctx.enter_context(tc.tile_pool(name="smp", bufs=CFG.get("sm_bufs", 2)))
        _attn_phase(
            tc, nc, plans, qT, kTt, vsb, wo_sb, tri, mts,
            sp_pool, cx_pool, ob_pool, ppool, c2pool, otpool, smpool, outT,
        )


def _qkv_phase(
    tc, nc, xTr, xpool, pps, tps, wq_sb, wk_sb, wv_sb, qT, kTt, vsb, ident, kTo, vTo
):
    for tt in range(TOK // QT):
        xt = xpool.tile([P, CS, QT], BF16, tag="xt")
        nsplit = 8 if tt == 0 else CFG["xt_split"]
        step = CS // nsplit
        for s in range(nsplit):
            nc.sync.dma_start(
                xt[:, s * step : (s + 1) * step, :],
                xTr[:, s * step : (s + 1) * step, tt * QT : (tt + 1) * QT],
            )
        tsl = slice(tt * QT, (tt + 1) * QT)
        for wsb, dst, out32 in (
            (wq_sb, qT, None),
            (wk_sb, kTt, kTo),
            (wv_sb, None, vTo),
        ):
            ps = pps.tile([P, QT], F32, tag="proj")
            for cs in range(CS):
                nc.tensor.matmul(
                    ps,
                    lhsT=wsb[:, cs, :],
                    rhs=xt[:, cs, :],
                    start=(cs == 0),
                    stop=(cs == CS - 1),
                )
            if dst is not None:
                nc.vector.tensor_copy(dst[:, tsl], ps)
                if out32 is not None:
                    nc.sync.dma_start(out32[:, tsl], dst[:, tsl])
            else:
                vt = xpool.tile([P, QT], BF16, tag="vt")
                nc.vector.tensor_copy(vt, ps)
                if out32 is not None:
                    nc.sync.dma_start(out32[:, tsl], vt)
                for sub in range(QT // P):
                    g = tt * (QT // P) + sub
                    tp = tps.tile([P, P], BF16, tag="tp")
                    nc.tensor.transpose(tp, vt[:, sub * P : (sub + 1) * P], ident)
                    nc.vector.tensor_copy(vsb[:, g, 0:D], tp[:, 0:D])
                    nc.vector.tensor_copy(vsb[:, g, VW : VW + D], tp[:, D:HD])


def _attn_phase(
    tc, nc, plans, qT, kTt, vsb, wo_sb, tri, mts,
    sp_pool, cx_pool, ob_pool, ppool, c2pool, otpool, smpool, outT,
):
    order = [(b, qi) for b in range(B) for qi in range(len(plans))]
    if CFG.get("qt_order") == "desc":
        order = [(b, qi) for b in range(B) for qi in reversed(range(len(plans)))]
    elif CFG.get("qt_order") == "mix":
        order = []
        for b in range(B):
            idx = list(range(len(plans)))
            hi, lo = idx[len(idx) // 2 :][::-1], idx[: len(idx) // 2]
            for a_, b_ in zip(hi, lo):
                order += [(b, a_), (b, b_)]
    for b, qi in order:
            ents = plans[qi]
            qoff = b * T + qi * QT
            cx = [
                cx_pool.tile([D + 2, QT], F32, tag=f"cx{h}", name=f"cx{h}")
                for h in range(HPC)
            ]
            n_e = len(ents)
            if n_e == 0:
                for h in range(HPC):
                    nc.vector.memset(cx[h], 0.0)
            for ei, (ki, qlo, j, mi) in enumerate(ents):
                g = b * (T // KT) + ki
                koff = g * KT
                sp = sp_pool.tile([P, HPC, QT], F32, tag="s", name="s")
                for h in range(HPC):
                    hs = h * D
                    nc.tensor.matmul(
                        sp[:, h, qlo:QT],
                        lhsT=kTt[hs : hs + D, koff : koff + KT],
                        rhs=qT[hs : hs + D, qoff + qlo : qoff + QT],
                        start=True,
                        stop=True,
                    )
                pt = ppool.tile([P, HPC, QT], BF16, tag="p", name="p")
                nc.scalar.activation(
                    pt[:, :, qlo:QT], sp[:, :, qlo:QT], EXP, scale=SCALE
                )
                for h in range(HPC):
                    if j is not None:
                        nc.gpsimd.tensor_mul(
                            pt[:, h, qlo : qlo + P], pt[:, h, qlo : qlo + P], tri
                        )
                    elif mi is not None:
                        nc.gpsimd.tensor_mul(pt[:, h, :], pt[:, h, :], mts[:, mi, :])
                    nc.tensor.matmul(
                        cx[h][:, qlo:QT],
                        lhsT=vsb[:, g, h * VW : (h + 1) * VW],
                        rhs=pt[:, h, qlo:QT],
                        start=(ei == 0),
                        stop=(ei == n_e - 1),
                        skip_group_check=True,
                    )

            c2 = c2pool.tile([P, QT], BF16, tag="c2")
            cxs = smpool.tile([D + 1, HPC, QT], F32, tag="cxs", name="cxs")
            for h in range(HPC):
                nc.vector.tensor_copy(cxs[:, h, :], cx[h][0 : D + 1, :])
            for h in range(HPC):
                rcp = smpool.tile([1, QT], F32, tag=f"r{h}", name=f"r{h}")
                nc.vector.reciprocal(rcp, cxs[D : D + 1, h, :])
                rb = smpool.tile([D, QT], F32, tag=f"rb{h}", name=f"rb{h}")
                nc.gpsimd.partition_broadcast(rb, rcp)
                nc.vector.tensor_mul(c2[h * D : (h + 1) * D, :], cxs[0:D, h, :], rb)
            ot = otpool.tile([P, CS, QT], F32, tag="ot")
            for os_ in range(C // P):
                op = ob_pool.tile([P, QT], F32, tag="ob", name="ob")
                nc.tensor.matmul(
                    op,
                    lhsT=wo_sb[:, os_ * P : (os_ + 1) * P],
                    rhs=c2,
                    start=True,
                    stop=True,
                )
                nc.vector.tensor_copy(ot[:, os_, :], op)
            oTr = outT.rearrange("(n p) m -> p n m", p=P)
            hcs = CS // 2
            if CFG.get("ot_split2"):
                nc.sync.dma_start(oTr[:, 0:hcs, qoff : qoff + QT], ot[:, 0:hcs, :])
                nc.sync.dma_start(oTr[:, hcs:CS, qoff : qoff + QT], ot[:, hcs:CS, :])
            else:
                nc.sync.dma_start(oTr[:, :, qoff : qoff + QT], ot)


def _build(plans, n_mt):
    nc = bacc.Bacc("TRN2", target_bir_lowering=False, debug=False)
    xT = nc.dram_tensor("xT", [C, TOK], BF16, kind="ExternalInput").ap()
    wq = nc.dram_tensor("wq", [C, HD], BF16, kind="ExternalInput").ap()
    wk = nc.dram_tensor("wk", [C, HD], BF16, kind="ExternalInput").ap()
    wv = nc.dram_tensor("wv", [C, HD], BF16, kind="ExternalInput").ap()
    wo = nc.dram_tensor("wo", [HD, C], BF16, kind="ExternalInput").ap()
    mt = nc.dram_tensor("mt", [n_mt, KT, QT], BF16, kind="ExternalInput").ap()
    tri_in = nc.dram_tensor("tri", [P, P], BF16, kind="ExternalInput").ap()
    outT = nc.dram_tensor("outT", [C, TOK], F32, kind="ExternalOutput").ap()
    kTo = nc.dram_tensor("kT", [HD, TOK], BF16, kind="ExternalOutput").ap()
    vTo = nc.dram_tensor("vT", [HD, TOK], BF16, kind="ExternalOutput").ap()
    with tile.TileContext(nc) as tc:
        _body(tc, nc, plans, n_mt, xT, wq, wk, wv, wo, mt, tri_in, outT, kTo, vTo)
    nc.compile()
    return nc


_CACHE = {}


def _get_nc(plans_key, plans, n_mt):
    if plans_key not in _CACHE:
        _CACHE[plans_key] = _build(plans, n_mt)
    return _CACHE[plans_key]


def _bf16(a):
    return np.asarray(a, np.float32).astype(ml_dtypes.bfloat16)


def _make_in_maps(x, w_qkv, w_o, mt):
    xT = _bf16(np.ascontiguousarray(x.reshape(TOK, C).T))
    tri = _bf16(np.triu(np.ones((P, P), np.float32)))  # tri[k, q] = 1 iff k <= q
    mtb = _bf16(mt)
    in_maps = []
    for c in range(NCORE):
        lo, hi = c * HD, (c + 1) * HD
        in_maps.append(
            {
                "xT": xT,
                "wq": _bf16(w_qkv[:, lo:hi]),
                "wk": _bf16(w_qkv[:, C + lo : C + hi]),
                "wv": _bf16(w_qkv[:, 2 * C + lo : 2 * C + hi]),
                "wo": _bf16(w_o[lo:hi, :]),
                "mt": mtb,
                "tri": tri,
            }
        )
    return in_maps


def _gather(results):
    out = np.zeros((C, TOK), np.float32)
    for r in results:
        out += r["outT"]
    out = np.ascontiguousarray(out.T).reshape(B, T, C)

    k = np.empty((B, H, T, D), np.float32)
    v = np.empty((B, H, T, D), np.float32)
    for c, r in enumerate(results):
        k[:, c * HPC : (c + 1) * HPC] = (
            r["kT"].astype(np.float32).reshape(HPC, D, B, T).transpose(2, 0, 3, 1)
        )
        v[:, c * HPC : (c + 1) * HPC] = (
            r["vT"].astype(np.float32).reshape(HPC, D, B, T).transpose(2, 0, 3, 1)
        )
    return out, k, v


def run(x, w_qkv, w_o, mask, trace=False):
    mask = np.asarray(mask)
    mask2d = np.asarray(np.broadcast_to(mask, (1, 1, T, T))).reshape(T, T).astype(bool)
    plans, mt = _plan(mask2d)
    plans_key = mask2d.tobytes()
    nc = _get_nc(plans_key, plans, mt.shape[0])
    in_maps = _make_in_maps(
        np.asarray(x, np.float32), np.asarray(w_qkv, np.float32),
        np.asarray(w_o, np.float32), mt,
    )
    res = run_bass_kernel_spmd(nc, in_maps, list(range(NCORE)), trace=trace)
    return _gather(res.results), res


def kernel(x, w_qkv, w_o, mask):
    (out, k, v), _ = run(x, w_qkv, w_o, mask)
    return out, k, v
